# revision 7
# baseline (speedup 1.0000x reference)
"""Multi-head attention (B=8, S=1024, D=1024, H=16) on 8 TRN2 NeuronCores.

Sharding: data-parallel over the batch dim - core b computes batch element b
end-to-end (projections + attention + output projection). No collectives.

Differences vs the padded baseline (299 us):
  - Scores use PE row-tiling: the two heads of an e-chunk run as K=64
    matmuls at tile_position (0,0) and (64,0) CONCURRENTLY (auto-derived
    from operand base partitions), so scores cost ~27 us instead of ~55.
  - ctx^T uses PE col-tiling: V_h (M=64) at col-position 0 and V_h' (M=64)
    at col-position 64 write rows 0:64 / 64:128 of one PSUM bank
    concurrently -> ctx ~27 us instead of ~55.
  - Softmax denominators come from separate M=1 ones-vector matmuls, four
    packed per PE slot at col positions {0,32,64,96} of one PSUM bank
    (rows: 0=(h,ih0), 32=(h',ih0), 64=(h,ih1), 96=(h',ih1)).
    PSUM 'start' clears has_written for the whole 2KB bank, so only the
    FIRST matmul of each bank-group uses start=True and only the LAST uses
    stop=True; interleaved chains in one bank then work correctly.
  - Q/K projections for pair p+1 and the V projection are emitted as PE
    filler inside pair p's attention loop so ScalarE's ~147 us of exp
    overlaps PE work instead of serializing.
  - Normalization: reciprocal_approx_fast on the denominator bank, rows
    copied to partition 0, GpSimd partition_broadcast to 64 rows, one DVE
    multiply per (head, ih) straight out of the ctx PSUM bank.

PSUM budget (8 banks): scores [128,1024] bufs=1 -> 2, proj [128,512]
bufs=2 -> 2, ctx [128,512] bufs=3 -> 3, denom [128,512] bufs=1 -> 1.
"""

import numpy as np
import ml_dtypes

import concourse.bass as bass
import concourse.mybir as mybir
import concourse.tile as tile
from concourse import bacc
from concourse.bass_utils import run_bass_kernel_spmd

BF = ml_dtypes.bfloat16

B, S, D, H = 8, 1024, 1024, 16
DK = D // H            # 64
P = 128
KT = D // P            # 8 contraction chunks
ET = D // P            # 8 e-tiles
ST = S // P            # 8 s/j tiles
FREE = 512             # one PSUM bank of fp32
NIH = S // FREE        # 2 i-halves
NPAIR = H // 2         # 8 head pairs (pair p <-> heads 2p, 2p+1, e-chunk p)
N_CORES = 8

F32 = mybir.dt.float32
BF16 = mybir.dt.bfloat16
ADD = mybir.AluOpType.add
MULT = mybir.AluOpType.mult
EXP = mybir.ActivationFunctionType.Exp
SCALE = float(1.0 / np.sqrt(DK))


def build_nc(repeat: int = 1):
    """Build + compile the SPMD single-core program (same NEFF on all cores)."""
    nc = bacc.Bacc("TRN2", target_bir_lowering=False, debug=False,
                   num_devices=N_CORES)

    xq_d = nc.dram_tensor("xq_t", [D, S], BF16, kind="ExternalInput")
    xk_d = nc.dram_tensor("xk_t", [D, S], BF16, kind="ExternalInput")
    xv_d = nc.dram_tensor("xv_t", [D, S], BF16, kind="ExternalInput")
    wq_d = nc.dram_tensor("wq_t", [D, D], BF16, kind="ExternalInput")
    wk_d = nc.dram_tensor("wk_t", [D, D], BF16, kind="ExternalInput")
    wv_d = nc.dram_tensor("wv_t", [D, D], BF16, kind="ExternalInput")
    wo_d = nc.dram_tensor("wo_t", [D, D], BF16, kind="ExternalInput")
    bq_d = nc.dram_tensor("bq_r", [P, ET], F32, kind="ExternalInput")
    bk_d = nc.dram_tensor("bk_r", [P, ET], F32, kind="ExternalInput")
    bvb_d = nc.dram_tensor("bvb", [P, D], F32, kind="ExternalInput")
    bob_d = nc.dram_tensor("bob", [P, D], F32, kind="ExternalInput")
    out_d = nc.dram_tensor("out", [S, D], F32, kind="ExternalOutput")

    with tile.TileContext(nc) as tc:
        with tc.tile_pool(name="xin", bufs=3 * KT) as xin, \
             tc.tile_pool(name="wgt", bufs=3 * KT) as wgt, \
             tc.tile_pool(name="qk", bufs=2 * ET) as qkp, \
             tc.tile_pool(name="vsb", bufs=ST) as vsb, \
             tc.tile_pool(name="att", bufs=5) as att, \
             tc.tile_pool(name="ctx", bufs=ET) as ctxp, \
             tc.tile_pool(name="outp", bufs=2) as outp, \
             tc.tile_pool(name="rcpp", bufs=2) as rcpp, \
             tc.tile_pool(name="d0p", bufs=2) as d0p, \
             tc.tile_pool(name="rbp", bufs=3) as rbp, \
             tc.tile_pool(name="cst", bufs=1) as cst, \
             tc.tile_pool(name="sc", bufs=1, space="PSUM") as scp, \
             tc.tile_pool(name="pj", bufs=2, space="PSUM") as pjp, \
             tc.tile_pool(name="cx", bufs=3, space="PSUM") as cxp, \
             tc.tile_pool(name="dn", bufs=1, space="PSUM") as dnp:

            # ---- constants (outside the repeat loop) ----
            bq_sb = cst.tile([P, ET], F32, name="bq_sb")
            bk_sb = cst.tile([P, ET], F32, name="bk_sb")
            bvb_sb = cst.tile([P, D], F32, name="bvb_sb")
            bob_sb = cst.tile([P, D], F32, name="bob_sb")
            ones_sb = cst.tile([P, 1], BF16, name="ones_sb")
            nc.sync.dma_start(out=bq_sb[:], in_=bq_d[:])
            nc.sync.dma_start(out=bk_sb[:], in_=bk_d[:])
            nc.sync.dma_start(out=bvb_sb[:], in_=bvb_d[:])
            nc.sync.dma_start(out=bob_sb[:], in_=bob_d[:])
            nc.vector.memset(ones_sb[:], 1.0)

            def body():
                # ---------- input DMA emission (lead) ----------
                xq_sb, xk_sb, wq_sb, wk_sb = [], [], [], []
                for k in range(KT):
                    t = xin.tile([P, S], BF16, tag="x", name=f"xq{k}")
                    nc.sync.dma_start(out=t[:], in_=xq_d[k * P:(k + 1) * P, :])
                    xq_sb.append(t)
                for k in range(KT):
                    t = xin.tile([P, S], BF16, tag="x", name=f"xk{k}")
                    nc.sync.dma_start(out=t[:], in_=xk_d[k * P:(k + 1) * P, :])
                    xk_sb.append(t)
                for k in range(KT):
                    t = wgt.tile([P, D], BF16, tag="w", name=f"wq{k}")
                    nc.sync.dma_start(out=t[:], in_=wq_d[k * P:(k + 1) * P, :])
                    wq_sb.append(t)
                for k in range(KT):
                    t = wgt.tile([P, D], BF16, tag="w", name=f"wk{k}")
                    nc.sync.dma_start(out=t[:], in_=wk_d[k * P:(k + 1) * P, :])
                    wk_sb.append(t)
                xv_sb, wv_sb, wo_sb = [], [], []

                def dma_xv():
                    for k in range(KT):
                        t = xin.tile([P, S], BF16, tag="x", name=f"xv{k}")
                        nc.sync.dma_start(out=t[:],
                                          in_=xv_d[k * P:(k + 1) * P, :])
                        xv_sb.append(t)
                        t = wgt.tile([P, D], BF16, tag="w", name=f"wv{k}")
                        nc.sync.dma_start(out=t[:],
                                          in_=wv_d[k * P:(k + 1) * P, :])
                        wv_sb.append(t)

                def dma_wo():
                    for k in range(KT):
                        t = wgt.tile([P, D], BF16, tag="w", name=f"wo{k}")
                        nc.sync.dma_start(out=t[:],
                                          in_=wo_d[k * P:(k + 1) * P, :])
                        wo_sb.append(t)

                qt_sb = [None] * ET   # Q^T [e, s] bf16, e on partitions
                kt_sb = [None] * ET   # K^T [e, s] bf16
                v_sb = [None] * ST    # V   [s, e] bf16, natural layout
                ctxt_sb = [ctxp.tile([P, S], BF16, tag="ctx",
                                     name=f"ctxt{p}") for p in range(NPAIR)]

                # ---------- filler units: one half-projection (8 MMs) ----------
                def qk_half(nm, et, ih):
                    """One i-half of the Q or K projection for e-chunk et."""
                    x_sb = xq_sb if nm == "q" else xk_sb
                    w_sb = wq_sb if nm == "q" else wk_sb
                    b_sb = bq_sb if nm == "q" else bk_sb
                    dst = qt_sb if nm == "q" else kt_sb
                    if dst[et] is None:
                        dst[et] = qkp.tile([P, S], BF16, tag="qk",
                                           name=f"{nm}t{et}")
                    psum = pjp.tile([P, FREE], F32, tag="pj",
                                    name=f"{nm}ps{et}_{ih}")
                    sl = slice(ih * FREE, (ih + 1) * FREE)
                    for k in range(KT):
                        nc.tensor.matmul(
                            psum[:],
                            w_sb[k][:, et * P:(et + 1) * P],
                            x_sb[k][:, sl],
                            start=(k == 0), stop=(k == KT - 1))
                    nc.vector.tensor_scalar(
                        out=dst[et][:, sl], in0=psum[:],
                        scalar1=b_sb[:, et:et + 1], scalar2=None, op0=ADD)

                def v_half(st, eh):
                    """One e-half of the V projection for s-chunk st."""
                    if v_sb[st] is None:
                        v_sb[st] = vsb.tile([P, D], BF16, tag="v",
                                            name=f"v{st}")
                    psum = pjp.tile([P, FREE], F32, tag="pj",
                                    name=f"vps{st}_{eh}")
                    sl = slice(eh * FREE, (eh + 1) * FREE)
                    for k in range(KT):
                        nc.tensor.matmul(
                            psum[:],
                            xv_sb[k][:, st * P:(st + 1) * P],
                            wv_sb[k][:, sl],
                            start=(k == 0), stop=(k == KT - 1))
                    nc.vector.tensor_tensor(
                        out=v_sb[st][:, sl], in0=psum[:], in1=bvb_sb[:, sl],
                        op=ADD)

                # per-pair filler schedules: list of 8 lists (one per jt step),
                # each a list of thunks to emit at that step.
                def filler_schedule(p):
                    units = []
                    if p == 0:
                        units.append(dma_xv)
                        for st in range(ST):
                            units.append(lambda st=st: v_half(st, 0))
                            units.append(lambda st=st: v_half(st, 1))
                        for ih in range(NIH):
                            units.append(lambda ih=ih: qk_half("q", 1, ih))
                        for ih in range(NIH):
                            units.append(lambda ih=ih: qk_half("k", 1, ih))
                    elif p <= 6:
                        et = p + 1
                        for ih in range(NIH):
                            units.append(
                                lambda ih=ih, et=et: qk_half("q", et, ih))
                        for ih in range(NIH):
                            units.append(
                                lambda ih=ih, et=et: qk_half("k", et, ih))
                        if p == 5:
                            units.append(dma_wo)
                    steps = [[] for _ in range(ST)]
                    for i, u in enumerate(units):
                        steps[(i * ST) // max(len(units), 1) % ST].append(u)
                    return steps

                # ---------- attention ----------
                def scores_ih(p, jt, ih):
                    """Row-tiled pair scores for one i-half -> exp -> attn tile.

                    sc tile [128,1024]: cols 0:512 head 2p, cols 512:1024 head
                    2p+1 (both for i-half ih). Returns the bf16 attn tile.
                    """
                    sc = scp.tile([P, 2 * FREE], F32, tag="sc",
                                  name=f"sc{p}_{jt}_{ih}")
                    isl = slice(ih * FREE, (ih + 1) * FREE)
                    jsl = slice(jt * P, (jt + 1) * P)
                    nc.tensor.matmul(sc[:, 0:FREE],
                                     kt_sb[p][0:DK, jsl],
                                     qt_sb[p][0:DK, isl],
                                     start=True, stop=True)
                    nc.tensor.matmul(sc[:, FREE:2 * FREE],
                                     kt_sb[p][DK:P, jsl],
                                     qt_sb[p][DK:P, isl],
                                     start=True, stop=True)
                    a = att.tile([P, 2 * FREE], BF16, tag="attn",
                                 name=f"attn{p}_{jt}_{ih}")
                    nc.scalar.activation(a[:], sc[:], EXP, scale=SCALE)
                    return a

                def ctx_dn(p, jt, cx, dn, attn):
                    """ctx col-tiled pair + packed denominator MMs for one jt.

                    cx: [cx_ih0, cx_ih1] PSUM tiles [128, 512]; rows 0:64 =
                    ctx^T head 2p, rows 64:128 = ctx^T head 2p+1.
                    dn: PSUM tile [128, 512]; rows 0/32/64/96 = denominators
                    for (h,ih0),(h',ih0),(h,ih1),(h',ih1).
                    attn: [attn_ih0, attn_ih1] bf16 tiles [128, 1024].
                    """
                    first = (jt == 0)
                    last = (jt == ST - 1)
                    vt = v_sb[jt]
                    # NB: on real HW, matmul start=True clears has_written
                    # only for the bytes it writes (per-element, NOT the
                    # whole bank as the sim models) - so every chain gets
                    # start=True on ITS first matmul.
                    for ih in range(NIH):
                        nc.tensor.matmul(
                            cx[ih][0:DK, :],
                            vt[:, p * P:p * P + DK],
                            attn[ih][:, 0:FREE],
                            start=first, stop=last,
                            skip_group_check=True)
                        nc.tensor.matmul(
                            cx[ih][DK:P, :],
                            vt[:, p * P + DK:(p + 1) * P],
                            attn[ih][:, FREE:2 * FREE],
                            start=first, stop=last,
                            skip_group_check=True)
                    for m in range(4):
                        ih, hh = m // 2, m % 2
                        nc.tensor.matmul(
                            dn[32 * m:32 * m + 1, :],
                            ones_sb[:],
                            attn[ih][:, hh * FREE:(hh + 1) * FREE],
                            start=first, stop=last,
                            skip_group_check=True,
                            tile_position=(0, 32 * m))

                def normalize(p, cx, dn):
                    """ctx^T / denom -> ctxt_sb[p] bf16 via recip + broadcast."""
                    dsb = rcpp.tile([P, FREE], F32, tag="dsb", name=f"dsb{p}")
                    nc.vector.tensor_copy(out=dsb[:], in_=dn[:])
                    rcp = rcpp.tile([P, FREE], F32, tag="rcp", name=f"rcp{p}")
                    nc.vector.reciprocal_approx_fast(out=rcp[:], in_=dsb[:])
                    # move denom rows 32/64/96 to partition 0 for broadcast
                    rows = [rcp]
                    for m in range(1, 4):
                        d0 = d0p.tile([1, FREE], F32, tag="d0",
                                      name=f"d0_{p}_{m}")
                        nc.vector.tensor_copy(out=d0[:],
                                              in_=rcp[32 * m:32 * m + 1, :])
                        rows.append(d0)
                    for m in range(4):
                        ih, hh = m // 2, m % 2
                        rb = rbp.tile([DK, FREE], F32, tag="rb",
                                      name=f"rb{p}_{m}")
                        nc.gpsimd.partition_broadcast(rb[:], rows[m][0:1, :])
                        nc.vector.tensor_tensor(
                            out=ctxt_sb[p][hh * DK:(hh + 1) * DK,
                                           ih * FREE:(ih + 1) * FREE],
                            in0=cx[ih][hh * DK:(hh + 1) * DK, :],
                            in1=rb[:], op=MULT)

                # ---------- emission schedule ----------
                # lead-in: Q/K projections for e-chunk 0
                for ih in range(NIH):
                    qk_half("q", 0, ih)
                for ih in range(NIH):
                    qk_half("k", 0, ih)

                for p in range(NPAIR):
                    steps = filler_schedule(p)
                    cx = [cxp.tile([P, FREE], F32, tag="cx",
                                   name=f"cx{p}_{ih}") for ih in range(NIH)]
                    dn = dnp.tile([P, FREE], F32, tag="dn", name=f"dn{p}")
                    prev_attn = None
                    for jt in range(ST):
                        a0 = scores_ih(p, jt, 0)
                        if prev_attn is not None:
                            ctx_dn(p, jt - 1, cx, dn, prev_attn)
                        for u in steps[jt][:len(steps[jt]) // 2]:
                            u()
                        a1 = scores_ih(p, jt, 1)
                        for u in steps[jt][len(steps[jt]) // 2:]:
                            u()
                        prev_attn = (a0, a1)
                    ctx_dn(p, ST - 1, cx, dn, prev_attn)
                    normalize(p, cx, dn)

                # ---------- output projection ----------
                for st in range(ST):
                    psum = [pjp.tile([P, FREE], F32, tag="pj",
                                     name=f"ops{st}_{eh}")
                            for eh in range(NIH)]
                    for eh in range(NIH):
                        esl = slice(eh * FREE, (eh + 1) * FREE)
                        for k in range(KT):
                            nc.tensor.matmul(
                                psum[eh][:],
                                ctxt_sb[k][:, st * P:(st + 1) * P],
                                wo_sb[k][:, esl],
                                start=(k == 0), stop=(k == KT - 1))
                    o = outp.tile([P, D], F32, tag="o", name=f"o{st}")
                    for eh in range(NIH):
                        esl = slice(eh * FREE, (eh + 1) * FREE)
                        nc.vector.tensor_tensor(out=o[:, esl],
                                                in0=psum[eh][:],
                                                in1=bob_sb[:, esl], op=ADD)
                    nc.sync.dma_start(out=out_d[st * P:(st + 1) * P, :],
                                      in_=o[:])

            if repeat == 1:
                body()
            else:
                with tc.For_i(0, repeat, 1) as _:
                    body()

    nc.compile()
    return nc


_NC_CACHE: dict = {}


def get_nc(repeat: int = 1):
    if repeat not in _NC_CACHE:
        _NC_CACHE[repeat] = build_nc(repeat)
    return _NC_CACHE[repeat]


def make_in_maps(query, key_, value, w_q, b_q, w_k, b_k, w_v, b_v, w_o, b_o):
    shared = {
        "wq_t": np.ascontiguousarray(np.asarray(w_q, np.float32).T).astype(BF),
        "wk_t": np.ascontiguousarray(np.asarray(w_k, np.float32).T).astype(BF),
        "wv_t": np.ascontiguousarray(np.asarray(w_v, np.float32).T).astype(BF),
        "wo_t": np.ascontiguousarray(np.asarray(w_o, np.float32).T).astype(BF),
        "bq_r": np.ascontiguousarray(
            np.asarray(b_q, np.float32).reshape(ET, P).T),
        "bk_r": np.ascontiguousarray(
            np.asarray(b_k, np.float32).reshape(ET, P).T),
        "bvb": np.ascontiguousarray(
            np.tile(np.asarray(b_v, np.float32)[None, :], (P, 1))),
        "bob": np.ascontiguousarray(
            np.tile(np.asarray(b_o, np.float32)[None, :], (P, 1))),
    }
    q = np.asarray(query, np.float32)
    k = np.asarray(key_, np.float32)
    v = np.asarray(value, np.float32)
    in_maps = []
    for b in range(B):
        m = dict(shared)
        m["xq_t"] = np.ascontiguousarray(q[b].T).astype(BF)
        m["xk_t"] = np.ascontiguousarray(k[b].T).astype(BF)
        m["xv_t"] = np.ascontiguousarray(v[b].T).astype(BF)
        in_maps.append(m)
    return in_maps


def run(in_maps, repeat: int = 1):
    nc = get_nc(repeat)
    res = run_bass_kernel_spmd(nc, in_maps, list(range(N_CORES)))
    return np.stack([np.asarray(res.results[i]["out"], np.float32)
                     for i in range(B)])


def kernel(query, key_, value, w_q, b_q, w_k, b_k, w_v, b_v, w_o, b_o):
    in_maps = make_in_maps(query, key_, value, w_q, b_q, w_k, b_k,
                           w_v, b_v, w_o, b_o)
    return run(in_maps, repeat=1)


if __name__ == "__main__":
    rng = np.random.default_rng(0)
    sc = 1.0 / np.sqrt(D)
    inputs = dict(
        query=rng.standard_normal((B, S, D), dtype=np.float32),
        key_=rng.standard_normal((B, S, D), dtype=np.float32),
        value=rng.standard_normal((B, S, D), dtype=np.float32),
        w_q=rng.standard_normal((D, D), dtype=np.float32) * sc,
        b_q=np.zeros(D, np.float32),
        w_k=rng.standard_normal((D, D), dtype=np.float32) * sc,
        b_k=np.zeros(D, np.float32),
        w_v=rng.standard_normal((D, D), dtype=np.float32) * sc,
        b_v=np.zeros(D, np.float32),
        w_o=rng.standard_normal((D, D), dtype=np.float32) * sc,
        b_o=np.zeros(D, np.float32),
    )
    out = kernel(**inputs)
    print("out", out.shape, out.dtype, float(np.abs(out).max()))
